# revision 8
# baseline (speedup 1.0000x reference)
"""Multi-head attention (B=2, S=2048, D=1024, H=16) on 8 Trainium2 cores.

Sharding: data-parallel over batch (2) x tensor-parallel over heads (16 -> 4
per core). Core c handles batch c//4, heads 4*(c%4) .. 4*(c%4)+3. Each core
computes its heads' Q/K/V projections (column-sliced weights), flash-style
attention with transposed-score layout, and a partial output projection
(row-sliced Wo). Host sums the 4 partials per batch and adds bv@Wo + bo.

v2 layout notes (vs the v1 baseline):
  - x tiles are DMA'd on the sync queue BEFORE the weights (which ride the
    scalar-engine HWDGE queue), so the first transpose starts ~2us in
    instead of ~25us.
  - x^T is built with regular f32r matmuls against identity (LDW-pipelined,
    HAM-warming) instead of transpose-mode (which is latency-bound at
    ~275ns/tile and doesn't count as PE-busy).
  - One PSUM pool with two 2-bank tags shared by both phases (no
    pool-close drain barrier between projection and attention phases):
      short: transpose staging / q|k projection pairs / scores / final-proj
      long : v-projection / attention accumulators (po)
  - Scores for the next unit are emitted inside the current unit's j-loop
    (depth-2 pipeline), so the ACT engine's exp stream never stalls at unit
    boundaries; the unit-finish work (bc + output projection) is deferred
    by two j-steps into the next unit.
  - The softmax-denominator reciprocal uses reciprocal_approx_fast (~18
    correct bits, ~5x faster than DVE reciprocal).
  - bc (the rank-1 1/sum broadcast) writes into partitions 64:128 of the
    just-drained po accumulator bank instead of a fresh PSUM tile.
  - Softmax denominators ride as the 65th row of the AV matmul (va has a
    ones column); matmuls run f32r / bf16.
"""

import numpy as np

B, S, D, H, DK = 2, 2048, 1024, 16, 64
HPC = 4          # heads per core
HD = HPC * DK    # 256 projected dims per core
P = 128
NB = 512
NCORES = 8

_CACHE = {}


def _install_tile_drain_fix():
    """TileContext._drain_and_barrier piles every outstanding sem wait onto
    one Drain instruction; this walrus build rejects >1 sync wait per
    instruction. Split the extra waits across single-wait NOPs."""
    import concourse.tile as tile
    from concourse.vector_clock import ScopedClock

    if getattr(tile.TileContext, "_ant_drain_fix", False):
        return

    def _drain_and_barrier_split(self, tick_clock, wait_clock):
        drain_inst = self.nc.sync.drain()
        wait_clock.add_sem_waits(
            drain_inst.ins, ScopedClock({None: tick_clock.global_clock})
        )
        waits = list(drain_inst.ins.sync_info.on_wait or [])
        if len(waits) > 1:
            drain_inst.ins.sync_info.on_wait = waits[:1]
            for w in waits[1:]:
                n = self.nc.sync.nop(nofuse=True)
                si = n.ins.sync_info
                if si is None:
                    import bass_rust

                    n.ins.sync_info = bass_rust.SyncInfo(on_wait=[w], on_update=[])
                else:
                    si.on_wait = [w]

        self.nc.all_engine_barrier()
        assert self.sems is not None
        popped = self.nc._tile_sem_poison_stack.pop()
        assert popped is self._sem_poison
        self.nc.clear_and_free_semaphores(list(self.sems.allocated().values()))
        self.nc.all_engine_barrier()

    tile.TileContext._drain_and_barrier = _drain_and_barrier_split
    tile.TileContext._ant_drain_fix = True


def _split_excess_waits(nc):
    """walrus's per-struct sync-wait capacity is small (observed: 1 for the
    self-loading-weight Matmult S3_LW struct, 2 for TPB_CTRL/Drain). Tile's
    wait assignment can leave many waits on one instruction; hoist the excess
    onto NOPs on the same engine immediately before it."""
    import concourse.mybir as mybir

    nid = [0]
    for f in nc.m.functions:
        for bb in f.blocks:
            out = []
            changed = False
            for inst in bb.instructions:
                si = getattr(inst, "sync_info", None)
                waits = list(si.on_wait) if si is not None and si.on_wait else []
                cap = 1
                if len(waits) > cap:
                    extra = waits[cap:]
                    for k in range(0, len(extra), 2):
                        nid[0] += 1
                        out.append(
                            mybir.InstEventSemaphore(
                                name=f"I-waitsplit-{nid[0]}",
                                ins=[],
                                outs=[],
                                sync_info=mybir.SyncInfo(
                                    on_wait=extra[k:k + 2], on_update=[]
                                ),
                                engine=inst.engine,
                            )
                        )
                    si.on_wait = waits[:cap]
                    changed = True
                out.append(inst)
            if changed:
                bb.instructions = out


def _build_program():
    import concourse.bass as bass
    import concourse.mybir as mybir
    from concourse.masks import make_identity
    from concourse.tile import TileContext

    _install_tile_drain_fix()

    f32 = mybir.dt.float32
    f32r = mybir.dt.float32r
    bf16 = mybir.dt.bfloat16
    Exp = mybir.ActivationFunctionType.Exp

    nc = bass.Bass()

    # x declared f32r so the identity-transpose matmuls run at full PE rate
    xb = nc.dram_tensor("xb", [S, D], f32r, kind="ExternalInput")
    wq = nc.dram_tensor("wq", [D, HD], f32r, kind="ExternalInput")
    wk = nc.dram_tensor("wk", [D, HD], f32r, kind="ExternalInput")
    wv = nc.dram_tensor("wv", [D, HD], f32r, kind="ExternalInput")
    wo = nc.dram_tensor("wo", [HD, D], f32r, kind="ExternalInput")
    bqt = nc.dram_tensor("bqt", [P, 2], f32, kind="ExternalInput")
    bkt = nc.dram_tensor("bkt", [P, 2], f32, kind="ExternalInput")
    outp = nc.dram_tensor("outp", [S, D], f32, kind="ExternalOutput")

    NDC = D // P      # 8 d-chunks
    NST = S // P      # 16 sequence tiles
    NSB = S // NB     # 4 sequence blocks

    with TileContext(nc) as tc:
        with tc.tile_pool(name="consts", bufs=1) as consts:
            ident = consts.tile([P, P], f32)
            make_identity(nc, ident)
            identr = consts.tile([P, P], f32r)
            nc.vector.tensor_copy(out=identr[:], in_=ident[:])
            # memset on a float32r AP emits invalid ISA; write the f32 bit
            # pattern of 1.0 through a uint32 view instead
            onesg = consts.tile([33, DK], f32r)
            nc.vector.memset(onesg.bitcast(mybir.dt.uint32), 0x3F800000)
            # pre-warm the ACT exp table set (~2.7us) while DMAs run
            warm = consts.tile([1, 2], f32)
            nc.scalar.activation(warm[0:1, 0:1], ident[0:1, 0:1], Exp)

            # weights ride the scalar-engine HWDGE queue so x tiles (sync
            # queue) aren't serialized behind 4MB of weights
            wv_sb = consts.tile([P, NDC, HD], f32r)
            nc.scalar.dma_start(wv_sb[:], wv.rearrange("(c p) h -> p c h", p=P))
            wq_sb = consts.tile([P, NDC, HD], f32r)
            nc.scalar.dma_start(wq_sb[:], wq.rearrange("(c p) h -> p c h", p=P))
            wk_sb = consts.tile([P, NDC, HD], f32r)
            nc.scalar.dma_start(wk_sb[:], wk.rearrange("(c p) h -> p c h", p=P))
            bq_sb = consts.tile([P, 2], f32)
            nc.scalar.dma_start(bq_sb[:], bqt[:])
            bk_sb = consts.tile([P, 2], f32)
            nc.scalar.dma_start(bk_sb[:], bkt[:])
            wo_sb = consts.tile([P, 2, D], f32r)
            nc.scalar.dma_start(wo_sb[:], wo.rearrange("(c p) d -> p c d", p=P))

            with (
                tc.tile_pool(name="acts", bufs=1) as acts,
                tc.tile_pool(name="ps", bufs=1, space="PSUM") as ps,
            ):
                xT = acts.tile([P, NDC, S], f32r)
                # pair-packed transposed projections: [2 heads x 64, S]
                qT = acts.tile([P, 2, S], f32r)
                kT = acts.tile([P, 2, S], f32r)
                # v augmented with a ones column (row 65 of the AV matmul
                # accumulates the softmax denominator): [s, j-tile, head, 65]
                va = acts.tile([P, NST, HPC, DK + 1], bf16)
                nc.vector.memset(va.bitcast(mybir.dt.uint16), 0x3F80)
                # Wo lhsT: [head-dim pair-chunk, pair, i]
                stack = acts.tile([P, 2, S], f32r)
                # softmax denominators for two heads at partitions 0 and 32;
                # filler rows preset to 1.0 so approx-recip never sees junk
                sums_sb = acts.tile([33, NB], f32)
                nc.vector.memset(sums_sb[:], 1.0)

                def short(nm):
                    return ps.tile([P, 2 * NB], f32, tag="short", bufs=2, name=nm)

                def long_(nm):
                    return ps.tile([P, 2 * NB], f32, tag="long", bufs=2, name=nm)

                # ---------------- phase 1: transposes + projections --------
                for it in range(NST):
                    xr = acts.tile([P, D], f32r, tag="xr", bufs=3, name=f"xr{it}")
                    nc.sync.dma_start(xr[:], xb[it * P:(it + 1) * P, :])
                    tr = short(f"tr{it}")
                    trv = tr.rearrange("p (c s) -> p c s", c=NDC)
                    for dd in range(NDC):
                        nc.tensor.matmul(
                            trv[:, dd, :], xr[:, dd * P:(dd + 1) * P], identr[:]
                        )
                    nc.vector.tensor_copy(
                        out=xT[:, :, it * P:(it + 1) * P], in_=trv[:]
                    )
                    vp = long_(f"vp{it}")
                    for d in range(NDC):
                        nc.tensor.matmul(
                            vp[:, 0:HD],
                            xT[:, d, it * P:(it + 1) * P],
                            wv_sb[:, d, :],
                            start=(d == 0),
                            stop=(d == NDC - 1),
                        )
                    nc.vector.tensor_copy(
                        out=va[:, it, :, 0:DK],
                        in_=vp[:, 0:HD].rearrange("p (h e) -> p h e", h=HPC),
                    )

                    if it % 4 != 3:
                        continue
                    sb = it // 4
                    for p in range(2):
                        pq = short(f"pq{sb}_{p}")
                        for col, w_sb in ((0, wq_sb), (NB, wk_sb)):
                            for d in range(NDC):
                                nc.tensor.matmul(
                                    pq[:, col:col + NB],
                                    w_sb[:, d, p * P:(p + 1) * P],
                                    xT[:, d, sb * NB:(sb + 1) * NB],
                                    start=(d == 0),
                                    stop=(d == NDC - 1),
                                )
                        nc.vector.tensor_scalar_add(
                            out=qT[:, p, sb * NB:(sb + 1) * NB],
                            in0=pq[:, 0:NB],
                            scalar1=bq_sb[:, p:p + 1],
                        )
                        nc.vector.tensor_scalar_add(
                            out=kT[:, p, sb * NB:(sb + 1) * NB],
                            in0=pq[:, NB:2 * NB],
                            scalar1=bk_sb[:, p:p + 1],
                        )

                # ---------------- phase 2: attention ------------------------
                units = [(ib, p) for ib in range(NSB) for p in range(2)]

                def emit_scores(u, j):
                    ib, p = units[u]
                    i0 = ib * NB
                    sc = short(f"sc{u}_{j}")
                    nc.tensor.matmul(
                        sc[:, 0:NB],
                        kT[0:DK, p, j * P:(j + 1) * P],
                        qT[0:DK, p, i0:i0 + NB],
                        tile_position=(0, 0),
                    )
                    nc.tensor.matmul(
                        sc[:, NB:2 * NB],
                        kT[DK:2 * DK, p, j * P:(j + 1) * P],
                        qT[DK:2 * DK, p, i0:i0 + NB],
                        tile_position=(64, 0),
                    )
                    return sc

                def emit_finish(u, po, rcr, po_sbs):
                    # bc: broadcast each head's 1/sumexp across 64 partitions
                    # via rank-1 matmul into the drained po bank's upper half,
                    # then scale the raw AV numerators into the Wo lhsT
                    ib, p = units[u]
                    i0 = ib * NB
                    bct = short(f"bc{u}")
                    for h in range(2):
                        nc.tensor.matmul(
                            bct[0:DK, h * NB:(h + 1) * NB],
                            onesg[32 * h:32 * h + 1, :],
                            rcr[32 * h:32 * h + 1, :],
                            tile_position=(32 * h, 0),
                        )
                        nc.vector.tensor_tensor(
                            out=stack[h * DK:(h + 1) * DK, p, i0:i0 + NB],
                            in0=po_sbs[h][:],
                            in1=bct[0:DK, h * NB:(h + 1) * NB],
                            op=mybir.AluOpType.mult,
                        )
                    if p != 1:
                        return
                    # final projection for this i-block (4 row tiles)
                    for t in range(NB // P):
                        it = ib * (NB // P) + t
                        fin = short(f"fin{it}")
                        for nbi in range(2):
                            for pch in range(2):
                                nc.tensor.matmul(
                                    fin[:, nbi * NB:(nbi + 1) * NB],
                                    stack[:, pch, it * P:(it + 1) * P],
                                    wo_sb[:, pch, nbi * NB:(nbi + 1) * NB],
                                    start=(pch == 0),
                                    stop=(pch == 1),
                                )
                        ot = acts.tile(
                            [P, D], f32, tag="ot", bufs=3, name=f"ot{it}"
                        )
                        nc.vector.tensor_copy(out=ot[:], in_=fin[:])
                        nc.sync.dma_start(outp[it * P:(it + 1) * P, :], ot[:])

                from collections import deque

                sc_q = deque()
                sc_q.append(emit_scores(0, 0))
                sc_q.append(emit_scores(0, 1))
                pending = None
                for u in range(8):
                    ib, p = units[u]
                    po = long_(f"po{u}")
                    for j in range(NST):
                        sc = sc_q.popleft()
                        ex = acts.tile(
                            [P, 2 * NB], bf16, tag="ex", bufs=3, name=f"ex{u}_{j}"
                        )
                        nc.scalar.activation(ex[:], sc[:], Exp, scale=0.125)
                        nj = j + 2
                        if nj < NST:
                            sc_q.append(emit_scores(u, nj))
                        elif u + 1 < 8:
                            sc_q.append(emit_scores(u + 1, nj - NST))
                        for h in range(2):
                            nc.tensor.matmul(
                                po[0:DK + 1, h * NB:(h + 1) * NB],
                                va[:, j, 2 * p + h, :],
                                ex[:, h * NB:(h + 1) * NB],
                                start=(j == 0),
                                stop=(j == NST - 1),
                            )
                        # deferred finish of the previous unit, 2 j-steps in
                        # (lets the DVE recip chain complete off PE's path)
                        if j == 1 and pending is not None:
                            emit_finish(*pending)
                            pending = None
                    # drain accumulators + denominators, fast reciprocal
                    po_sbs = []
                    for h in range(2):
                        po_sb = acts.tile(
                            [DK, NB], f32, tag="posb", bufs=4, name=f"posb{u}_{h}"
                        )
                        nc.vector.tensor_copy(
                            out=po_sb[:], in_=po[0:DK, h * NB:(h + 1) * NB]
                        )
                        po_sbs.append(po_sb)
                        nc.vector.tensor_copy(
                            out=sums_sb[32 * h:32 * h + 1, :],
                            in_=po[DK:DK + 1, h * NB:(h + 1) * NB],
                        )
                    rcr = acts.tile(
                        [33, NB], f32r, tag="rcr", bufs=2, name=f"rcr{u}"
                    )
                    with nc.allow_low_precision("fp22 recip feeds f32r matmul"):
                        nc.vector.reciprocal(out=rcr[:], in_=sums_sb[:])
                    pending = (u, po, rcr, po_sbs)
                emit_finish(*pending)

    _split_excess_waits(nc)
    return nc


def _get_program():
    if "nc" not in _CACHE:
        _CACHE["nc"] = _build_program()
    return _CACHE["nc"]


def kernel(x, Wq, bq, Wk, bk, Wv, bv, Wo, bo, _trace=False):
    from concourse.bass_utils import run_bass_kernel_spmd

    x = np.asarray(x, dtype=np.float32)
    Wq = np.asarray(Wq, dtype=np.float32)
    Wk = np.asarray(Wk, dtype=np.float32)
    Wv = np.asarray(Wv, dtype=np.float32)
    Wo = np.asarray(Wo, dtype=np.float32)
    bq = np.asarray(bq, dtype=np.float32)
    bk = np.asarray(bk, dtype=np.float32)
    bv = np.asarray(bv, dtype=np.float32)
    bo = np.asarray(bo, dtype=np.float32)

    in_maps = []
    for c in range(NCORES):
        b = c // 4
        cs = (c % 4) * HD
        in_maps.append({
            "xb": np.ascontiguousarray(x[b]),
            "wq": np.ascontiguousarray(Wq[:, cs:cs + HD]),
            "wk": np.ascontiguousarray(Wk[:, cs:cs + HD]),
            "wv": np.ascontiguousarray(Wv[:, cs:cs + HD]),
            "wo": np.ascontiguousarray(Wo[cs:cs + HD, :]),
            "bqt": np.ascontiguousarray(bq[cs:cs + HD].reshape(2, P).T),
            "bkt": np.ascontiguousarray(bk[cs:cs + HD].reshape(2, P).T),
        })

    nc = _get_program()
    res = run_bass_kernel_spmd(
        nc, in_maps, core_ids=list(range(NCORES)), trace=_trace
    )

    cvec = (bv @ Wo + bo).astype(np.float32)
    out = np.empty((B, S, D), dtype=np.float32)
    for b in range(B):
        acc = res.results[4 * b]["outp"].astype(np.float64)
        for c in range(4 * b + 1, 4 * b + 4):
            acc = acc + res.results[c]["outp"]
        out[b] = (acc + cvec).astype(np.float32)

    if _trace:
        _CACHE["last_results"] = res
    return out


# revision 10
# speedup vs baseline: 1.1290x; 1.1290x over previous
"""Multi-head attention (B=2, S=2048, D=1024, H=16) on 8 Trainium2 cores.

Sharding: data-parallel over batch (2) x tensor-parallel over heads (16 -> 4
per core). Core c handles batch c//4, heads 4*(c%4) .. 4*(c%4)+3. Each core
computes its heads' Q/K/V projections (column-sliced weights), flash-style
attention with transposed-score layout, and a partial output projection
(row-sliced Wo). Host sums the 4 partials per batch and adds bv@Wo + bo.

v2 layout notes (vs the v1 baseline):
  - x tiles are DMA'd on the sync queue BEFORE the weights (which ride the
    scalar-engine HWDGE queue), so the first transpose starts ~2us in
    instead of ~25us.
  - x^T is built with regular f32r matmuls against identity (LDW-pipelined,
    HAM-warming) instead of transpose-mode (which is latency-bound at
    ~275ns/tile and doesn't count as PE-busy).
  - One PSUM pool with two 2-bank tags shared by both phases (no
    pool-close drain barrier between projection and attention phases):
      short: transpose staging / q|k projection pairs / scores / final-proj
      long : v-projection / attention accumulators (po)
  - Scores for the next unit are emitted inside the current unit's j-loop
    (depth-2 pipeline), so the ACT engine's exp stream never stalls at unit
    boundaries; the unit-finish work (bc + output projection) is deferred
    by two j-steps into the next unit.
  - The softmax-denominator reciprocal uses reciprocal_approx_fast (~18
    correct bits, ~5x faster than DVE reciprocal).
  - bc (the rank-1 1/sum broadcast) writes into partitions 64:128 of the
    just-drained po accumulator bank instead of a fresh PSUM tile.
  - Softmax denominators ride as the 65th row of the AV matmul (va has a
    ones column); matmuls run f32r / bf16.
"""

import numpy as np

B, S, D, H, DK = 2, 2048, 1024, 16, 64
HPC = 4          # heads per core
HD = HPC * DK    # 256 projected dims per core
P = 128
NB = 512
NCORES = 8

_CACHE = {}


def _install_tile_drain_fix():
    """TileContext._drain_and_barrier piles every outstanding sem wait onto
    one Drain instruction; this walrus build rejects >1 sync wait per
    instruction. Split the extra waits across single-wait NOPs."""
    import concourse.tile as tile
    from concourse.vector_clock import ScopedClock

    if getattr(tile.TileContext, "_ant_drain_fix", False):
        return

    def _drain_and_barrier_split(self, tick_clock, wait_clock):
        drain_inst = self.nc.sync.drain()
        wait_clock.add_sem_waits(
            drain_inst.ins, ScopedClock({None: tick_clock.global_clock})
        )
        waits = list(drain_inst.ins.sync_info.on_wait or [])
        if len(waits) > 1:
            drain_inst.ins.sync_info.on_wait = waits[:1]
            for w in waits[1:]:
                n = self.nc.sync.nop(nofuse=True)
                si = n.ins.sync_info
                if si is None:
                    import bass_rust

                    n.ins.sync_info = bass_rust.SyncInfo(on_wait=[w], on_update=[])
                else:
                    si.on_wait = [w]

        self.nc.all_engine_barrier()
        assert self.sems is not None
        popped = self.nc._tile_sem_poison_stack.pop()
        assert popped is self._sem_poison
        self.nc.clear_and_free_semaphores(list(self.sems.allocated().values()))
        self.nc.all_engine_barrier()

    tile.TileContext._drain_and_barrier = _drain_and_barrier_split
    tile.TileContext._ant_drain_fix = True


def _split_excess_waits(nc):
    """walrus's per-struct sync-wait capacity is small (observed: 1 for the
    self-loading-weight Matmult S3_LW struct, 2 for TPB_CTRL/Drain). Tile's
    wait assignment can leave many waits on one instruction; hoist the excess
    onto NOPs on the same engine immediately before it."""
    import concourse.mybir as mybir

    nid = [0]
    for f in nc.m.functions:
        for bb in f.blocks:
            out = []
            changed = False
            for inst in bb.instructions:
                si = getattr(inst, "sync_info", None)
                waits = list(si.on_wait) if si is not None and si.on_wait else []
                cap = 1
                if len(waits) > cap:
                    extra = waits[cap:]
                    for k in range(0, len(extra), 2):
                        nid[0] += 1
                        out.append(
                            mybir.InstEventSemaphore(
                                name=f"I-waitsplit-{nid[0]}",
                                ins=[],
                                outs=[],
                                sync_info=mybir.SyncInfo(
                                    on_wait=extra[k:k + 2], on_update=[]
                                ),
                                engine=inst.engine,
                            )
                        )
                    si.on_wait = waits[:cap]
                    changed = True
                out.append(inst)
            if changed:
                bb.instructions = out


def _build_program():
    import concourse.bass as bass
    import concourse.mybir as mybir
    from concourse.masks import make_identity
    from concourse.tile import TileContext

    _install_tile_drain_fix()

    f32 = mybir.dt.float32
    f32r = mybir.dt.float32r
    bf16 = mybir.dt.bfloat16
    Exp = mybir.ActivationFunctionType.Exp

    nc = bass.Bass()

    # x declared f32r so the identity-transpose matmuls run at full PE rate
    xb = nc.dram_tensor("xb", [S, D], f32r, kind="ExternalInput")
    wq = nc.dram_tensor("wq", [D, HD], f32r, kind="ExternalInput")
    wk = nc.dram_tensor("wk", [D, HD], f32r, kind="ExternalInput")
    wv = nc.dram_tensor("wv", [D, HD], f32r, kind="ExternalInput")
    wo = nc.dram_tensor("wo", [HD, D], f32r, kind="ExternalInput")
    bqt = nc.dram_tensor("bqt", [P, 2], f32, kind="ExternalInput")
    bkt = nc.dram_tensor("bkt", [P, 2], f32, kind="ExternalInput")
    outp = nc.dram_tensor("outp", [S, D], f32, kind="ExternalOutput")

    NDC = D // P      # 8 d-chunks
    NST = S // P      # 16 sequence tiles
    NSB = S // NB     # 4 sequence blocks

    with TileContext(nc) as tc:
        with tc.tile_pool(name="consts", bufs=1) as consts:
            ident = consts.tile([P, P], f32)
            make_identity(nc, ident)
            identr = consts.tile([P, P], f32r)
            nc.vector.tensor_copy(out=identr[:], in_=ident[:])
            # memset on a float32r AP emits invalid ISA; write the f32 bit
            # pattern of 1.0 through a uint32 view instead
            onesg = consts.tile([33, DK], f32r)
            nc.vector.memset(onesg.bitcast(mybir.dt.uint32), 0x3F800000)
            # pre-warm the ACT exp table set (~2.7us) while DMAs run
            warm = consts.tile([1, 2], f32)
            nc.scalar.activation(warm[0:1, 0:1], ident[0:1, 0:1], Exp)

            # weights ride the scalar-engine HWDGE queue so x tiles (sync
            # queue) aren't serialized behind 4MB of weights
            wv_sb = consts.tile([P, NDC, HD], f32r)
            nc.scalar.dma_start(wv_sb[:], wv.rearrange("(c p) h -> p c h", p=P))
            wq_sb = consts.tile([P, NDC, HD], f32r)
            nc.scalar.dma_start(wq_sb[:], wq.rearrange("(c p) h -> p c h", p=P))
            wk_sb = consts.tile([P, NDC, HD], f32r)
            nc.scalar.dma_start(wk_sb[:], wk.rearrange("(c p) h -> p c h", p=P))
            bq_sb = consts.tile([P, 2], f32)
            nc.scalar.dma_start(bq_sb[:], bqt[:])
            bk_sb = consts.tile([P, 2], f32)
            nc.scalar.dma_start(bk_sb[:], bkt[:])
            wo_sb = consts.tile([P, 2, D], f32r)
            nc.scalar.dma_start(wo_sb[:], wo.rearrange("(c p) d -> p c d", p=P))

            with (
                tc.tile_pool(name="acts", bufs=1) as acts,
                tc.tile_pool(name="ps", bufs=1, space="PSUM") as ps,
            ):
                xT = acts.tile([P, NDC, S], f32r)
                # pair-packed transposed projections: [2 heads x 64, S]
                qT = acts.tile([P, 2, S], f32r)
                kT = acts.tile([P, 2, S], f32r)
                # v augmented with a ones column (row 65 of the AV matmul
                # accumulates the softmax denominator): [s, j-tile, head, 65]
                va = acts.tile([P, NST, HPC, DK + 1], bf16)
                nc.vector.memset(va.bitcast(mybir.dt.uint16), 0x3F80)
                # Wo lhsT: [head-dim pair-chunk, pair, i]
                stack = acts.tile([P, 2, S], f32r)
                # softmax denominators for two heads at partitions 0 and 32;
                # filler rows preset to 1.0 so approx-recip never sees junk
                sums_sb = acts.tile([33, NB], f32)
                nc.vector.memset(sums_sb[:], 1.0)

                def short(nm):
                    return ps.tile([P, 2 * NB], f32, tag="short", bufs=2, name=nm)

                def long_(nm):
                    return ps.tile([P, 2 * NB], f32, tag="long", bufs=1, name=nm)

                def fint(nm):
                    return ps.tile([P, 2 * NB], f32, tag="fin", bufs=1, name=nm)

                # ---------------- phase 1: transposes + projections --------
                # software-pipelined: transposes of tile it+1 are emitted
                # before the v-projection of tile it, hiding the DVE
                # PSUM->SBUF xT copy latency behind PE work
                def emit_transp(it):
                    xr = acts.tile([P, D], f32r, tag="xr", bufs=3, name=f"xr{it}")
                    nc.sync.dma_start(xr[:], xb[it * P:(it + 1) * P, :])
                    tr = short(f"tr{it}")
                    trv = tr.rearrange("p (c s) -> p c s", c=NDC)
                    for dd in range(NDC):
                        nc.tensor.matmul(
                            trv[:, dd, :], xr[:, dd * P:(dd + 1) * P], identr[:]
                        )
                    nc.vector.tensor_copy(
                        out=xT[:, :, it * P:(it + 1) * P], in_=trv[:]
                    )

                def emit_vproj(it):
                    vp = long_(f"vp{it}")
                    for d in range(NDC):
                        nc.tensor.matmul(
                            vp[:, 0:HD],
                            xT[:, d, it * P:(it + 1) * P],
                            wv_sb[:, d, :],
                            start=(d == 0),
                            stop=(d == NDC - 1),
                        )
                    nc.vector.tensor_copy(
                        out=va[:, it, :, 0:DK],
                        in_=vp[:, 0:HD].rearrange("p (h e) -> p h e", h=HPC),
                    )

                def emit_qk(sb):
                    for p in range(2):
                        pq = short(f"pq{sb}_{p}")
                        for col, w_sb in ((0, wq_sb), (NB, wk_sb)):
                            for d in range(NDC):
                                nc.tensor.matmul(
                                    pq[:, col:col + NB],
                                    w_sb[:, d, p * P:(p + 1) * P],
                                    xT[:, d, sb * NB:(sb + 1) * NB],
                                    start=(d == 0),
                                    stop=(d == NDC - 1),
                                )
                        nc.vector.tensor_scalar_add(
                            out=qT[:, p, sb * NB:(sb + 1) * NB],
                            in0=pq[:, 0:NB],
                            scalar1=bq_sb[:, p:p + 1],
                        )
                        nc.vector.tensor_scalar_add(
                            out=kT[:, p, sb * NB:(sb + 1) * NB],
                            in0=pq[:, NB:2 * NB],
                            scalar1=bk_sb[:, p:p + 1],
                        )

                # ---------------- phase 2 helpers ---------------------------
                units = [(ib, p) for ib in range(NSB) for p in range(2)]

                def emit_scores(u, j):
                    ib, p = units[u]
                    i0 = ib * NB
                    sc = short(f"sc{u}_{j}")
                    nc.tensor.matmul(
                        sc[:, 0:NB],
                        kT[0:DK, p, j * P:(j + 1) * P],
                        qT[0:DK, p, i0:i0 + NB],
                        tile_position=(0, 0),
                    )
                    nc.tensor.matmul(
                        sc[:, NB:2 * NB],
                        kT[DK:2 * DK, p, j * P:(j + 1) * P],
                        qT[DK:2 * DK, p, i0:i0 + NB],
                        tile_position=(64, 0),
                    )
                    return sc

                def emit_bc(u, rcr, po_sbs):
                    # bc: broadcast each head's 1/sumexp across 64 partitions
                    # via rank-1 matmul, then scale the raw AV numerators into
                    # the Wo lhsT
                    ib, p = units[u]
                    i0 = ib * NB
                    bct = fint(f"bc{u}")
                    for h in range(2):
                        nc.tensor.matmul(
                            bct[0:DK, h * NB:(h + 1) * NB],
                            onesg[32 * h:32 * h + 1, :],
                            rcr[32 * h:32 * h + 1, :],
                            tile_position=(32 * h, 0),
                        )
                        nc.vector.tensor_tensor(
                            out=stack[h * DK:(h + 1) * DK, p, i0:i0 + NB],
                            in0=po_sbs[h][:],
                            in1=bct[0:DK, h * NB:(h + 1) * NB],
                            op=mybir.AluOpType.mult,
                        )

                def emit_fin(ib, t, tail=False):
                    # output projection for row-tile t of i-block ib
                    it = ib * (NB // P) + t
                    fin = short(f"fin{it}") if (tail and t % 2) else fint(
                        f"fin{it}"
                    )
                    for nbi in range(2):
                        for pch in range(2):
                            nc.tensor.matmul(
                                fin[:, nbi * NB:(nbi + 1) * NB],
                                stack[:, pch, it * P:(it + 1) * P],
                                wo_sb[:, pch, nbi * NB:(nbi + 1) * NB],
                                start=(pch == 0),
                                stop=(pch == 1),
                            )
                    ot = acts.tile([P, D], f32, tag="ot", bufs=3, name=f"ot{it}")
                    nc.vector.tensor_copy(out=ot[:], in_=fin[:])
                    nc.sync.dma_start(outp[it * P:(it + 1) * P, :], ot[:])

                from collections import deque

                sc_q = deque()

                # ---------------- phase 1 emission --------------------------
                emit_transp(0)
                for it in range(NST):
                    if it + 1 < NST:
                        emit_transp(it + 1)
                    else:
                        # fill the final-tile copy latency with the first two
                        # score tiles so the exp stream starts during phase 1
                        sc_q.append(emit_scores(0, 0))
                        sc_q.append(emit_scores(0, 1))
                    emit_vproj(it)
                    if it % 4 == 3:
                        emit_qk(it // 4)

                # ---------------- phase 2 emission --------------------------
                # finish work of unit u-1 (bc + output projection) is spread
                # over unit u's j-loop at points where its DVE dependencies
                # (drain+reciprocal chain) are already satisfied, so it never
                # head-of-line-blocks the score matmuls feeding ACT
                pending = None
                for u in range(8):
                    ib, p = units[u]
                    po = long_(f"po{u}")
                    for j in range(NST):
                        sc = sc_q.popleft()
                        ex = acts.tile(
                            [P, 2 * NB], bf16, tag="ex", bufs=3, name=f"ex{u}_{j}"
                        )
                        nc.scalar.activation(ex[:], sc[:], Exp, scale=0.125)
                        nj = j + 2
                        if nj < NST:
                            sc_q.append(emit_scores(u, nj))
                        elif u + 1 < 8:
                            sc_q.append(emit_scores(u + 1, nj - NST))
                        for h in range(2):
                            nc.tensor.matmul(
                                po[0:DK + 1, h * NB:(h + 1) * NB],
                                va[:, j, 2 * p + h, :],
                                ex[:, h * NB:(h + 1) * NB],
                                start=(j == 0),
                                stop=(j == NST - 1),
                            )
                        if pending is not None:
                            pu, rcr, po_sbs = pending
                            pp = units[pu][1]
                            if j == 5:
                                emit_bc(pu, rcr, po_sbs)
                            elif pp == 1 and j in (7, 9, 11, 13):
                                emit_fin(units[pu][0], (j - 7) // 2)
                    # drain accumulators + denominators + reciprocal (DVE)
                    po_sbs = []
                    for h in range(2):
                        po_sb = acts.tile(
                            [DK, NB], f32, tag="posb", bufs=4, name=f"posb{u}_{h}"
                        )
                        nc.vector.tensor_copy(
                            out=po_sb[:], in_=po[0:DK, h * NB:(h + 1) * NB]
                        )
                        po_sbs.append(po_sb)
                        nc.vector.tensor_copy(
                            out=sums_sb[32 * h:32 * h + 1, :],
                            in_=po[DK:DK + 1, h * NB:(h + 1) * NB],
                        )
                    rcr = acts.tile(
                        [33, NB], f32r, tag="rcr", bufs=2, name=f"rcr{u}"
                    )
                    with nc.allow_low_precision("fp22 recip feeds f32r matmul"):
                        nc.vector.reciprocal(out=rcr[:], in_=sums_sb[:])
                    pending = (u, rcr, po_sbs)
                # tail: finish of the last unit
                pu, rcr, po_sbs = pending
                emit_bc(pu, rcr, po_sbs)
                for t in range(NB // P):
                    emit_fin(units[pu][0], t, tail=True)

    _split_excess_waits(nc)
    return nc


def _get_program():
    if "nc" not in _CACHE:
        _CACHE["nc"] = _build_program()
    return _CACHE["nc"]


def kernel(x, Wq, bq, Wk, bk, Wv, bv, Wo, bo, _trace=False):
    from concourse.bass_utils import run_bass_kernel_spmd

    x = np.asarray(x, dtype=np.float32)
    Wq = np.asarray(Wq, dtype=np.float32)
    Wk = np.asarray(Wk, dtype=np.float32)
    Wv = np.asarray(Wv, dtype=np.float32)
    Wo = np.asarray(Wo, dtype=np.float32)
    bq = np.asarray(bq, dtype=np.float32)
    bk = np.asarray(bk, dtype=np.float32)
    bv = np.asarray(bv, dtype=np.float32)
    bo = np.asarray(bo, dtype=np.float32)

    in_maps = []
    for c in range(NCORES):
        b = c // 4
        cs = (c % 4) * HD
        in_maps.append({
            "xb": np.ascontiguousarray(x[b]),
            "wq": np.ascontiguousarray(Wq[:, cs:cs + HD]),
            "wk": np.ascontiguousarray(Wk[:, cs:cs + HD]),
            "wv": np.ascontiguousarray(Wv[:, cs:cs + HD]),
            "wo": np.ascontiguousarray(Wo[cs:cs + HD, :]),
            "bqt": np.ascontiguousarray(bq[cs:cs + HD].reshape(2, P).T),
            "bkt": np.ascontiguousarray(bk[cs:cs + HD].reshape(2, P).T),
        })

    nc = _get_program()
    res = run_bass_kernel_spmd(
        nc, in_maps, core_ids=list(range(NCORES)), trace=_trace
    )

    cvec = (bv @ Wo + bo).astype(np.float32)
    out = np.empty((B, S, D), dtype=np.float32)
    for b in range(B):
        acc = res.results[4 * b]["outp"].astype(np.float64)
        for c in range(4 * b + 1, 4 * b + 4):
            acc = acc + res.results[c]["outp"]
        out[b] = (acc + cvec).astype(np.float32)

    if _trace:
        _CACHE["last_results"] = res
    return out


# revision 11
# speedup vs baseline: 1.3193x; 1.1685x over previous
"""Multi-head attention (B=2, S=2048, D=1024, H=16) on 8 Trainium2 cores.

Sharding: data-parallel over batch (2) x tensor-parallel over heads (16 -> 4
per core). Core c handles batch c//4, heads 4*(c%4) .. 4*(c%4)+3. Each core
computes its heads' Q/K/V projections (column-sliced weights), flash-style
attention with transposed-score layout, and a partial output projection
(row-sliced Wo). Host sums the 4 partials per batch and adds bv@Wo + bo.

v2 layout notes (vs the v1 baseline):
  - x tiles are DMA'd on the sync queue BEFORE the weights (which ride the
    scalar-engine HWDGE queue), so the first transpose starts ~2us in
    instead of ~25us.
  - x^T is built with regular f32r matmuls against identity (LDW-pipelined,
    HAM-warming) instead of transpose-mode (which is latency-bound at
    ~275ns/tile and doesn't count as PE-busy).
  - One PSUM pool with two 2-bank tags shared by both phases (no
    pool-close drain barrier between projection and attention phases):
      short: transpose staging / q|k projection pairs / scores / final-proj
      long : v-projection / attention accumulators (po)
  - Scores for the next unit are emitted inside the current unit's j-loop
    (depth-2 pipeline), so the ACT engine's exp stream never stalls at unit
    boundaries; the unit-finish work (bc + output projection) is deferred
    by two j-steps into the next unit.
  - The softmax-denominator reciprocal uses reciprocal_approx_fast (~18
    correct bits, ~5x faster than DVE reciprocal).
  - bc (the rank-1 1/sum broadcast) writes into partitions 64:128 of the
    just-drained po accumulator bank instead of a fresh PSUM tile.
  - Softmax denominators ride as the 65th row of the AV matmul (va has a
    ones column); matmuls run f32r / bf16.
"""

import numpy as np

B, S, D, H, DK = 2, 2048, 1024, 16, 64
HPC = 4          # heads per core
HD = HPC * DK    # 256 projected dims per core
P = 128
NB = 512
NCORES = 8

_CACHE = {}


def _install_tile_drain_fix():
    """TileContext._drain_and_barrier piles every outstanding sem wait onto
    one Drain instruction; this walrus build rejects >1 sync wait per
    instruction. Split the extra waits across single-wait NOPs."""
    import concourse.tile as tile
    from concourse.vector_clock import ScopedClock

    if getattr(tile.TileContext, "_ant_drain_fix", False):
        return

    def _drain_and_barrier_split(self, tick_clock, wait_clock):
        drain_inst = self.nc.sync.drain()
        wait_clock.add_sem_waits(
            drain_inst.ins, ScopedClock({None: tick_clock.global_clock})
        )
        waits = list(drain_inst.ins.sync_info.on_wait or [])
        if len(waits) > 1:
            drain_inst.ins.sync_info.on_wait = waits[:1]
            for w in waits[1:]:
                n = self.nc.sync.nop(nofuse=True)
                si = n.ins.sync_info
                if si is None:
                    import bass_rust

                    n.ins.sync_info = bass_rust.SyncInfo(on_wait=[w], on_update=[])
                else:
                    si.on_wait = [w]

        self.nc.all_engine_barrier()
        assert self.sems is not None
        popped = self.nc._tile_sem_poison_stack.pop()
        assert popped is self._sem_poison
        self.nc.clear_and_free_semaphores(list(self.sems.allocated().values()))
        self.nc.all_engine_barrier()

    tile.TileContext._drain_and_barrier = _drain_and_barrier_split
    tile.TileContext._ant_drain_fix = True


def _split_excess_waits(nc):
    """walrus's per-struct sync-wait capacity is small (observed: 1 for the
    self-loading-weight Matmult S3_LW struct, 2 for TPB_CTRL/Drain). Tile's
    wait assignment can leave many waits on one instruction; hoist the excess
    onto NOPs on the same engine immediately before it."""
    import concourse.mybir as mybir

    nid = [0]
    for f in nc.m.functions:
        for bb in f.blocks:
            out = []
            changed = False
            for inst in bb.instructions:
                si = getattr(inst, "sync_info", None)
                waits = list(si.on_wait) if si is not None and si.on_wait else []
                cap = 1
                if len(waits) > cap:
                    extra = waits[cap:]
                    for k in range(0, len(extra), 2):
                        nid[0] += 1
                        out.append(
                            mybir.InstEventSemaphore(
                                name=f"I-waitsplit-{nid[0]}",
                                ins=[],
                                outs=[],
                                sync_info=mybir.SyncInfo(
                                    on_wait=extra[k:k + 2], on_update=[]
                                ),
                                engine=inst.engine,
                            )
                        )
                    si.on_wait = waits[:cap]
                    changed = True
                out.append(inst)
            if changed:
                bb.instructions = out


def _build_program():
    import concourse.bass as bass
    import concourse.mybir as mybir
    from concourse.masks import make_identity
    from concourse.tile import TileContext

    _install_tile_drain_fix()

    f32 = mybir.dt.float32
    f32r = mybir.dt.float32r
    bf16 = mybir.dt.bfloat16
    Exp = mybir.ActivationFunctionType.Exp

    nc = bass.Bass()

    # x declared f32r so the identity-transpose matmuls run at full PE rate
    xb = nc.dram_tensor("xb", [S, D], f32r, kind="ExternalInput")
    wq = nc.dram_tensor("wq", [D, HD], f32r, kind="ExternalInput")
    wk = nc.dram_tensor("wk", [D, HD], f32r, kind="ExternalInput")
    wv = nc.dram_tensor("wv", [D, HD], f32r, kind="ExternalInput")
    wo = nc.dram_tensor("wo", [HD, D], f32r, kind="ExternalInput")
    bqt = nc.dram_tensor("bqt", [P, 2], f32, kind="ExternalInput")
    bkt = nc.dram_tensor("bkt", [P, 2], f32, kind="ExternalInput")
    outp = nc.dram_tensor("outp", [S, D], f32, kind="ExternalOutput")

    NDC = D // P      # 8 d-chunks
    NST = S // P      # 16 sequence tiles
    NSB = S // NB     # 4 sequence blocks

    with TileContext(nc) as tc:
        with tc.tile_pool(name="consts", bufs=1) as consts:
            ident = consts.tile([P, P], f32)
            make_identity(nc, ident)
            identr = consts.tile([P, P], f32r)
            nc.vector.tensor_copy(out=identr[:], in_=ident[:])
            # memset on a float32r AP emits invalid ISA; write the f32 bit
            # pattern of 1.0 through a uint32 view instead
            onesg = consts.tile([33, DK], f32r)
            nc.vector.memset(onesg.bitcast(mybir.dt.uint32), 0x3F800000)
            # pre-warm the ACT exp table set (~2.7us) while DMAs run.
            # NOTE all tile free-sizes here are 64B multiples — an odd-sized
            # tile mid-pool misaligns every later SBUF base and costs ~20%
            # on every ACT/DVE op and matmul operand read.
            warm = consts.tile([1, 16], f32)
            nc.scalar.activation(warm[0:1, 0:1], ident[0:1, 0:1], Exp)

            # weights ride the scalar-engine HWDGE queue so x tiles (sync
            # queue) aren't serialized behind 4MB of weights
            wv_sb = consts.tile([P, NDC, HD], f32r)
            nc.scalar.dma_start(wv_sb[:], wv.rearrange("(c p) h -> p c h", p=P))
            wq_sb = consts.tile([P, NDC, HD], f32r)
            nc.scalar.dma_start(wq_sb[:], wq.rearrange("(c p) h -> p c h", p=P))
            wk_sb = consts.tile([P, NDC, HD], f32r)
            nc.scalar.dma_start(wk_sb[:], wk.rearrange("(c p) h -> p c h", p=P))
            bq_sb = consts.tile([P, 16], f32)
            nc.scalar.dma_start(bq_sb[:, 0:2], bqt[:])
            bk_sb = consts.tile([P, 16], f32)
            nc.scalar.dma_start(bk_sb[:, 0:2], bkt[:])
            wo_sb = consts.tile([P, 2, D], f32r)
            nc.scalar.dma_start(wo_sb[:], wo.rearrange("(c p) d -> p c d", p=P))

            with (
                tc.tile_pool(name="acts", bufs=1) as acts,
                tc.tile_pool(name="ps", bufs=1, space="PSUM") as ps,
            ):
                xT = acts.tile([P, NDC, S], f32r)
                # pair-packed transposed projections: [2 heads x 64, S]
                qT = acts.tile([P, 2, S], f32r)
                kT = acts.tile([P, 2, S], f32r)
                # v augmented with a ones column (row 65 of the AV matmul
                # accumulates the softmax denominator): [s, j-tile, head, 65]
                va = acts.tile([P, NST, HPC, DK + 1], bf16)
                nc.vector.memset(va.bitcast(mybir.dt.uint16), 0x3F80)
                # Wo lhsT: [head-dim pair-chunk, pair, i]
                stack = acts.tile([P, 2, S], f32r)
                # softmax denominators for two heads at partitions 0 and 32;
                # filler rows preset to 1.0 so approx-recip never sees junk
                sums_sb = acts.tile([33, NB], f32)
                nc.vector.memset(sums_sb[:], 1.0)

                def short(nm):
                    return ps.tile([P, 2 * NB], f32, tag="short", bufs=2, name=nm)

                def long_(nm):
                    return ps.tile([P, 2 * NB], f32, tag="long", bufs=1, name=nm)

                def fint(nm):
                    return ps.tile([P, 2 * NB], f32, tag="fin", bufs=1, name=nm)

                # ---------------- phase 1: transposes + projections --------
                # software-pipelined: transposes of tile it+1 are emitted
                # before the v-projection of tile it, hiding the DVE
                # PSUM->SBUF xT copy latency behind PE work
                def emit_transp(it):
                    xr = acts.tile([P, D], f32r, tag="xr", bufs=3, name=f"xr{it}")
                    nc.sync.dma_start(xr[:], xb[it * P:(it + 1) * P, :])
                    tr = short(f"tr{it}")
                    trv = tr.rearrange("p (c s) -> p c s", c=NDC)
                    for dd in range(NDC):
                        nc.tensor.matmul(
                            trv[:, dd, :], xr[:, dd * P:(dd + 1) * P], identr[:]
                        )
                    nc.vector.tensor_copy(
                        out=xT[:, :, it * P:(it + 1) * P], in_=trv[:]
                    )

                def emit_vproj(it):
                    vp = long_(f"vp{it}")
                    for d in range(NDC):
                        nc.tensor.matmul(
                            vp[:, 0:HD],
                            xT[:, d, it * P:(it + 1) * P],
                            wv_sb[:, d, :],
                            start=(d == 0),
                            stop=(d == NDC - 1),
                        )
                    nc.vector.tensor_copy(
                        out=va[:, it, :, 0:DK],
                        in_=vp[:, 0:HD].rearrange("p (h e) -> p h e", h=HPC),
                    )

                def emit_qk(sb):
                    for p in range(2):
                        pq = short(f"pq{sb}_{p}")
                        for col, w_sb in ((0, wq_sb), (NB, wk_sb)):
                            for d in range(NDC):
                                nc.tensor.matmul(
                                    pq[:, col:col + NB],
                                    w_sb[:, d, p * P:(p + 1) * P],
                                    xT[:, d, sb * NB:(sb + 1) * NB],
                                    start=(d == 0),
                                    stop=(d == NDC - 1),
                                )
                        nc.vector.tensor_scalar_add(
                            out=qT[:, p, sb * NB:(sb + 1) * NB],
                            in0=pq[:, 0:NB],
                            scalar1=bq_sb[:, p:p + 1],
                        )
                        nc.vector.tensor_scalar_add(
                            out=kT[:, p, sb * NB:(sb + 1) * NB],
                            in0=pq[:, NB:2 * NB],
                            scalar1=bk_sb[:, p:p + 1],
                        )

                # ---------------- phase 2 helpers ---------------------------
                units = [(ib, p) for ib in range(NSB) for p in range(2)]

                def emit_scores(u, j):
                    ib, p = units[u]
                    i0 = ib * NB
                    sc = short(f"sc{u}_{j}")
                    nc.tensor.matmul(
                        sc[:, 0:NB],
                        kT[0:DK, p, j * P:(j + 1) * P],
                        qT[0:DK, p, i0:i0 + NB],
                        tile_position=(0, 0),
                    )
                    nc.tensor.matmul(
                        sc[:, NB:2 * NB],
                        kT[DK:2 * DK, p, j * P:(j + 1) * P],
                        qT[DK:2 * DK, p, i0:i0 + NB],
                        tile_position=(64, 0),
                    )
                    return sc

                def emit_bc(u, rcr, po_sbs):
                    # bc: broadcast each head's 1/sumexp across 64 partitions
                    # via rank-1 matmul, then scale the raw AV numerators into
                    # the Wo lhsT
                    ib, p = units[u]
                    i0 = ib * NB
                    bct = fint(f"bc{u}")
                    for h in range(2):
                        nc.tensor.matmul(
                            bct[0:DK, h * NB:(h + 1) * NB],
                            onesg[32 * h:32 * h + 1, :],
                            rcr[32 * h:32 * h + 1, :],
                            tile_position=(32 * h, 0),
                        )
                        nc.vector.tensor_tensor(
                            out=stack[h * DK:(h + 1) * DK, p, i0:i0 + NB],
                            in0=po_sbs[h][:],
                            in1=bct[0:DK, h * NB:(h + 1) * NB],
                            op=mybir.AluOpType.mult,
                        )

                def emit_fin(ib, t, tail=False):
                    # output projection for row-tile t of i-block ib
                    it = ib * (NB // P) + t
                    fin = short(f"fin{it}") if (tail and t % 2) else fint(
                        f"fin{it}"
                    )
                    for nbi in range(2):
                        for pch in range(2):
                            nc.tensor.matmul(
                                fin[:, nbi * NB:(nbi + 1) * NB],
                                stack[:, pch, it * P:(it + 1) * P],
                                wo_sb[:, pch, nbi * NB:(nbi + 1) * NB],
                                start=(pch == 0),
                                stop=(pch == 1),
                            )
                    ot = acts.tile([P, D], f32, tag="ot", bufs=3, name=f"ot{it}")
                    nc.vector.tensor_copy(out=ot[:], in_=fin[:])
                    nc.sync.dma_start(outp[it * P:(it + 1) * P, :], ot[:])

                from collections import deque

                sc_q = deque()

                # ---------------- phase 1 emission --------------------------
                emit_transp(0)
                for it in range(NST):
                    if it + 1 < NST:
                        emit_transp(it + 1)
                    else:
                        # fill the final-tile copy latency with the first two
                        # score tiles so the exp stream starts during phase 1
                        sc_q.append(emit_scores(0, 0))
                        sc_q.append(emit_scores(0, 1))
                    emit_vproj(it)
                    if it % 4 == 3:
                        emit_qk(it // 4)

                # ---------------- phase 2 emission --------------------------
                # finish work of unit u-1 (bc + output projection) is spread
                # over unit u's j-loop at points where its DVE dependencies
                # (drain+reciprocal chain) are already satisfied, so it never
                # head-of-line-blocks the score matmuls feeding ACT
                pending = None
                for u in range(8):
                    ib, p = units[u]
                    po = long_(f"po{u}")
                    for j in range(NST):
                        sc = sc_q.popleft()
                        ex = acts.tile(
                            [P, 2 * NB], bf16, tag="ex", bufs=3, name=f"ex{u}_{j}"
                        )
                        nc.scalar.activation(ex[:], sc[:], Exp, scale=0.125)
                        nj = j + 2
                        if nj < NST:
                            sc_q.append(emit_scores(u, nj))
                        elif u + 1 < 8:
                            sc_q.append(emit_scores(u + 1, nj - NST))
                        for h in range(2):
                            nc.tensor.matmul(
                                po[0:DK + 1, h * NB:(h + 1) * NB],
                                va[:, j, 2 * p + h, :],
                                ex[:, h * NB:(h + 1) * NB],
                                start=(j == 0),
                                stop=(j == NST - 1),
                            )
                        if pending is not None:
                            pu, rcr, po_sbs = pending
                            pp = units[pu][1]
                            if j == 5:
                                emit_bc(pu, rcr, po_sbs)
                            elif pp == 1 and j in (7, 9, 11, 13):
                                emit_fin(units[pu][0], (j - 7) // 2)
                    # drain accumulators + denominators + reciprocal (DVE)
                    po_sbs = []
                    for h in range(2):
                        po_sb = acts.tile(
                            [DK, NB], f32, tag="posb", bufs=4, name=f"posb{u}_{h}"
                        )
                        nc.vector.tensor_copy(
                            out=po_sb[:], in_=po[0:DK, h * NB:(h + 1) * NB]
                        )
                        po_sbs.append(po_sb)
                        nc.vector.tensor_copy(
                            out=sums_sb[32 * h:32 * h + 1, :],
                            in_=po[DK:DK + 1, h * NB:(h + 1) * NB],
                        )
                    rcr = acts.tile(
                        [33, NB], f32r, tag="rcr", bufs=2, name=f"rcr{u}"
                    )
                    with nc.allow_low_precision("fp22 recip feeds f32r matmul"):
                        nc.vector.reciprocal(out=rcr[:], in_=sums_sb[:])
                    pending = (u, rcr, po_sbs)
                # tail: finish of the last unit
                pu, rcr, po_sbs = pending
                emit_bc(pu, rcr, po_sbs)
                for t in range(NB // P):
                    emit_fin(units[pu][0], t, tail=True)

    _split_excess_waits(nc)
    return nc


def _get_program():
    if "nc" not in _CACHE:
        _CACHE["nc"] = _build_program()
    return _CACHE["nc"]


def kernel(x, Wq, bq, Wk, bk, Wv, bv, Wo, bo, _trace=False):
    from concourse.bass_utils import run_bass_kernel_spmd

    x = np.asarray(x, dtype=np.float32)
    Wq = np.asarray(Wq, dtype=np.float32)
    Wk = np.asarray(Wk, dtype=np.float32)
    Wv = np.asarray(Wv, dtype=np.float32)
    Wo = np.asarray(Wo, dtype=np.float32)
    bq = np.asarray(bq, dtype=np.float32)
    bk = np.asarray(bk, dtype=np.float32)
    bv = np.asarray(bv, dtype=np.float32)
    bo = np.asarray(bo, dtype=np.float32)

    in_maps = []
    for c in range(NCORES):
        b = c // 4
        cs = (c % 4) * HD
        in_maps.append({
            "xb": np.ascontiguousarray(x[b]),
            "wq": np.ascontiguousarray(Wq[:, cs:cs + HD]),
            "wk": np.ascontiguousarray(Wk[:, cs:cs + HD]),
            "wv": np.ascontiguousarray(Wv[:, cs:cs + HD]),
            "wo": np.ascontiguousarray(Wo[cs:cs + HD, :]),
            "bqt": np.ascontiguousarray(bq[cs:cs + HD].reshape(2, P).T),
            "bkt": np.ascontiguousarray(bk[cs:cs + HD].reshape(2, P).T),
        })

    nc = _get_program()
    res = run_bass_kernel_spmd(
        nc, in_maps, core_ids=list(range(NCORES)), trace=_trace
    )

    cvec = (bv @ Wo + bo).astype(np.float32)
    out = np.empty((B, S, D), dtype=np.float32)
    for b in range(B):
        acc = res.results[4 * b]["outp"].astype(np.float64)
        for c in range(4 * b + 1, 4 * b + 4):
            acc = acc + res.results[c]["outp"]
        out[b] = (acc + cvec).astype(np.float32)

    if _trace:
        _CACHE["last_results"] = res
    return out


# revision 12
# speedup vs baseline: 1.4649x; 1.1104x over previous
"""Multi-head attention (B=2, S=2048, D=1024, H=16) on 8 Trainium2 cores.

Sharding: data-parallel over batch (2) x tensor-parallel over heads (16 -> 4
per core). Core c handles batch c//4, heads 4*(c%4) .. 4*(c%4)+3. Each core
computes its heads' Q/K/V projections (column-sliced weights), flash-style
attention with transposed-score layout, and a partial output projection
(row-sliced Wo). Host sums the 4 partials per batch and adds bv@Wo + bo.

v5 design:
  - x and the Q/K/V weights are cast to bf16 on the host. x^T lands in SBUF
    via 32 DMA-XBAR transposes (16-bit only; ~14ns per 16x128 tile on the
    DMA engines) — no PE transposes, no PSUM->SBUF xT copies at all.
  - Q/K projections write bf16 qT/kT (pair-packed [2 heads x 64, S]);
    V-projection writes bf16 va augmented with a ones column whose AV row
    accumulates the softmax denominator.
  - Attention unit u = (i-block, head-pair): j-loop over 16 key tiles;
    scores (row-packed K=64 pair) -> exp on ACT (the true roofline:
    16.8M exps @ 1 elem/lane/cycle @ 1.2GHz ~= 110us) -> AV accumulation.
    Scores for the next unit are emitted inside the current unit's j-loop
    so ACT never stalls at unit boundaries.
  - Unit 0's first 10 j-steps are interleaved into phase 1 (lagging the
    projection stream by 6 tiles), so ACT starts ~40us earlier.
  - Finish work (1/sum broadcast + output projection row-tiles) is spread
    uniformly across subsequent units' j-loops (bc at j==5, one fin at
    j==8 and j==12) to keep the PE dense enough that HAM stays at K=8/8.
  - One PSUM pool, three tags shared by both phases (no pool-close drain):
      short [128,1024] x2: q|k projection pairs, scores
      small [128, 512] x2: v-projection, bc broadcast, out-proj halves
      long  [128,1024] x1: AV accumulators (po0|po1)
  - All SBUF tile free-sizes are 64B multiples: an odd-sized tile mid-pool
    misaligns every later tile's base and costs ~20% on every ACT/DVE op
    and matmul operand read.
"""

import numpy as np

B, S, D, H, DK = 2, 2048, 1024, 16, 64
HPC = 4          # heads per core
HD = HPC * DK    # 256 projected dims per core
P = 128
NB = 512
NCORES = 8

_CACHE = {}


def _install_tile_drain_fix():
    """TileContext._drain_and_barrier piles every outstanding sem wait onto
    one Drain instruction; this walrus build rejects >1 sync wait per
    instruction. Split the extra waits across single-wait NOPs."""
    import concourse.tile as tile
    from concourse.vector_clock import ScopedClock

    if getattr(tile.TileContext, "_ant_drain_fix", False):
        return

    def _drain_and_barrier_split(self, tick_clock, wait_clock):
        drain_inst = self.nc.sync.drain()
        wait_clock.add_sem_waits(
            drain_inst.ins, ScopedClock({None: tick_clock.global_clock})
        )
        waits = list(drain_inst.ins.sync_info.on_wait or [])
        if len(waits) > 1:
            drain_inst.ins.sync_info.on_wait = waits[:1]
            for w in waits[1:]:
                n = self.nc.sync.nop(nofuse=True)
                si = n.ins.sync_info
                if si is None:
                    import bass_rust

                    n.ins.sync_info = bass_rust.SyncInfo(on_wait=[w], on_update=[])
                else:
                    si.on_wait = [w]

        self.nc.all_engine_barrier()
        assert self.sems is not None
        popped = self.nc._tile_sem_poison_stack.pop()
        assert popped is self._sem_poison
        self.nc.clear_and_free_semaphores(list(self.sems.allocated().values()))
        self.nc.all_engine_barrier()

    tile.TileContext._drain_and_barrier = _drain_and_barrier_split
    tile.TileContext._ant_drain_fix = True


def _split_excess_waits(nc):
    """walrus's per-struct sync-wait capacity is small (observed: 1 for the
    self-loading-weight Matmult S3_LW struct, 2 for TPB_CTRL/Drain). Tile's
    wait assignment can leave many waits on one instruction; hoist the excess
    onto NOPs on the same engine immediately before it."""
    import concourse.mybir as mybir

    nid = [0]
    for f in nc.m.functions:
        for bb in f.blocks:
            out = []
            changed = False
            for inst in bb.instructions:
                si = getattr(inst, "sync_info", None)
                waits = list(si.on_wait) if si is not None and si.on_wait else []
                cap = 1
                if len(waits) > cap:
                    extra = waits[cap:]
                    for k in range(0, len(extra), 2):
                        nid[0] += 1
                        out.append(
                            mybir.InstEventSemaphore(
                                name=f"I-waitsplit-{nid[0]}",
                                ins=[],
                                outs=[],
                                sync_info=mybir.SyncInfo(
                                    on_wait=extra[k:k + 2], on_update=[]
                                ),
                                engine=inst.engine,
                            )
                        )
                    si.on_wait = waits[:cap]
                    changed = True
                out.append(inst)
            if changed:
                bb.instructions = out


def _build_program():
    import concourse.bass as bass
    import concourse.mybir as mybir
    from concourse.tile import TileContext

    _install_tile_drain_fix()

    f32 = mybir.dt.float32
    f32r = mybir.dt.float32r
    bf16 = mybir.dt.bfloat16
    Exp = mybir.ActivationFunctionType.Exp

    nc = bass.Bass()

    xbh = nc.dram_tensor("xbh", [S, D], bf16, kind="ExternalInput")
    wq = nc.dram_tensor("wq", [D, HD], bf16, kind="ExternalInput")
    wk = nc.dram_tensor("wk", [D, HD], bf16, kind="ExternalInput")
    wv = nc.dram_tensor("wv", [D, HD], bf16, kind="ExternalInput")
    wo = nc.dram_tensor("wo", [HD, D], f32r, kind="ExternalInput")
    bqt = nc.dram_tensor("bqt", [P, 2], f32, kind="ExternalInput")
    bkt = nc.dram_tensor("bkt", [P, 2], f32, kind="ExternalInput")
    outp = nc.dram_tensor("outp", [S, D], f32, kind="ExternalOutput")

    NDC = D // P      # 8 d-chunks
    NST = S // P      # 16 sequence tiles
    NSB = S // NB     # 4 sequence blocks

    with TileContext(nc) as tc:
        with tc.tile_pool(name="consts", bufs=1) as consts:
            onesg = consts.tile([33, DK], f32r)
            nc.vector.memset(onesg.bitcast(mybir.dt.uint32), 0x3F800000)
            # pre-warm the ACT exp table set (~2.7us) while DMAs run
            wconst = consts.tile([1, 16], f32)
            nc.vector.memset(wconst[:], 1.0)
            warm = consts.tile([1, 16], f32)
            nc.scalar.activation(warm[0:1, 0:1], wconst[0:1, 0:1], Exp)

            # weights ride the scalar-engine HWDGE queue so the x transposes
            # (sync queue) aren't serialized behind them
            wv_sb = consts.tile([P, NDC, HD], bf16)
            nc.scalar.dma_start(wv_sb[:], wv.rearrange("(c p) h -> p c h", p=P))
            wq_sb = consts.tile([P, NDC, HD], bf16)
            nc.scalar.dma_start(wq_sb[:], wq.rearrange("(c p) h -> p c h", p=P))
            wk_sb = consts.tile([P, NDC, HD], bf16)
            nc.scalar.dma_start(wk_sb[:], wk.rearrange("(c p) h -> p c h", p=P))
            bq_sb = consts.tile([P, 16], f32)
            nc.scalar.dma_start(bq_sb[:, 0:2], bqt[:])
            bk_sb = consts.tile([P, 16], f32)
            nc.scalar.dma_start(bk_sb[:, 0:2], bkt[:])
            wo_sb = consts.tile([P, 2, D], f32r)
            nc.scalar.dma_start(wo_sb[:], wo.rearrange("(c p) d -> p c d", p=P))

            with (
                tc.tile_pool(name="acts", bufs=1) as acts,
                tc.tile_pool(name="ps", bufs=1, space="PSUM") as ps,
            ):
                xT = acts.tile([P, NDC, S], bf16)
                # pair-packed transposed projections: [2 heads x 64, S]
                qT = acts.tile([P, 2, S], bf16)
                kT = acts.tile([P, 2, S], bf16)
                # v augmented with a ones column (row 65 of the AV matmul
                # accumulates the softmax denominator): [s, j-tile, head, 65]
                va = acts.tile([P, NST, HPC, DK + 1], bf16)
                nc.vector.memset(va.bitcast(mybir.dt.uint16), 0x3F80)
                # Wo lhsT: [head-dim pair-chunk, pair, i]
                stack = acts.tile([P, 2, S], f32r)
                # softmax denominators for two heads at partitions 0 and 32;
                # filler rows preset to 1.0 so reciprocal never sees junk
                sums_sb = acts.tile([33, NB], f32)
                nc.vector.memset(sums_sb[:], 1.0)

                # x^T via DMA-XBAR transposes, one per (s-block, d-chunk)
                for sb in range(NSB):
                    for c in range(NDC):
                        nc.sync.dma_start(
                            xT[:, c, sb * NB:(sb + 1) * NB],
                            xbh[sb * NB:(sb + 1) * NB, c * P:(c + 1) * P],
                            transpose=True,
                        )

                def short(nm):
                    return ps.tile([P, 2 * NB], f32, tag="short", bufs=2, name=nm)

                def small(nm):
                    return ps.tile([P, NB], f32, tag="small", bufs=2, name=nm)

                def long_(nm):
                    return ps.tile([P, 2 * NB], f32, tag="long", bufs=1, name=nm)

                # ---------------- projections -------------------------------
                def emit_vproj(it):
                    vp = small(f"vp{it}")
                    for d in range(NDC):
                        nc.tensor.matmul(
                            vp[:, 0:HD],
                            xT[:, d, it * P:(it + 1) * P],
                            wv_sb[:, d, :],
                            start=(d == 0),
                            stop=(d == NDC - 1),
                        )
                    nc.vector.tensor_copy(
                        out=va[:, it, :, 0:DK],
                        in_=vp[:, 0:HD].rearrange("p (h e) -> p h e", h=HPC),
                    )

                def emit_qk(sb):
                    for p in range(2):
                        pq = short(f"pq{sb}_{p}")
                        for col, w_sb in ((0, wq_sb), (NB, wk_sb)):
                            for d in range(NDC):
                                nc.tensor.matmul(
                                    pq[:, col:col + NB],
                                    w_sb[:, d, p * P:(p + 1) * P],
                                    xT[:, d, sb * NB:(sb + 1) * NB],
                                    start=(d == 0),
                                    stop=(d == NDC - 1),
                                )
                        with nc.allow_low_precision("bf16 q/k feed scores"):
                            nc.vector.tensor_scalar_add(
                                out=qT[:, p, sb * NB:(sb + 1) * NB],
                                in0=pq[:, 0:NB],
                                scalar1=bq_sb[:, p:p + 1],
                            )
                            nc.vector.tensor_scalar_add(
                                out=kT[:, p, sb * NB:(sb + 1) * NB],
                                in0=pq[:, NB:2 * NB],
                                scalar1=bk_sb[:, p:p + 1],
                            )

                # ---------------- attention helpers -------------------------
                units = [(ib, p) for ib in range(NSB) for p in range(2)]

                def emit_scores(u, j):
                    ib, p = units[u]
                    i0 = ib * NB
                    sc = short(f"sc{u}_{j}")
                    nc.tensor.matmul(
                        sc[:, 0:NB],
                        kT[0:DK, p, j * P:(j + 1) * P],
                        qT[0:DK, p, i0:i0 + NB],
                        tile_position=(0, 0),
                    )
                    nc.tensor.matmul(
                        sc[:, NB:2 * NB],
                        kT[DK:2 * DK, p, j * P:(j + 1) * P],
                        qT[DK:2 * DK, p, i0:i0 + NB],
                        tile_position=(64, 0),
                    )
                    return sc

                def emit_bc(u, rcr, po_sbs):
                    # broadcast each head's 1/sumexp across 64 partitions via
                    # rank-1 matmul, then scale the AV numerators into the
                    # Wo lhsT
                    ib, p = units[u]
                    i0 = ib * NB
                    for h in range(2):
                        bct = small(f"bc{u}_{h}")
                        nc.tensor.matmul(
                            bct[0:DK, :],
                            onesg[32 * h:32 * h + 1, :],
                            rcr[32 * h:32 * h + 1, :],
                            tile_position=(32 * h, 0),
                        )
                        nc.vector.tensor_tensor(
                            out=stack[h * DK:(h + 1) * DK, p, i0:i0 + NB],
                            in0=po_sbs[h][:],
                            in1=bct[0:DK, :],
                            op=mybir.AluOpType.mult,
                        )

                def emit_fin(ib, t):
                    # output projection for row-tile t of i-block ib, split
                    # into two D-halves on the 1-bank "small" ring
                    it = ib * (NB // P) + t
                    for nbi in range(2):
                        fin = small(f"fin{it}_{nbi}")
                        for pch in range(2):
                            nc.tensor.matmul(
                                fin[:],
                                stack[:, pch, it * P:(it + 1) * P],
                                wo_sb[:, pch, nbi * NB:(nbi + 1) * NB],
                                start=(pch == 0),
                                stop=(pch == 1),
                            )
                        ot = acts.tile(
                            [P, NB], f32, tag="ot", bufs=3, name=f"ot{it}_{nbi}"
                        )
                        nc.vector.tensor_copy(out=ot[:], in_=fin[:])
                        nc.sync.dma_start(
                            outp[it * P:(it + 1) * P, nbi * NB:(nbi + 1) * NB],
                            ot[:],
                        )

                from collections import deque

                sc_q = deque()
                fin_q = deque()
                state = {"pending": None, "po": None}

                def emit_unit_end(u):
                    # drain accumulators + denominators + reciprocal (DVE)
                    po = state["po"]
                    po_sbs = []
                    for h in range(2):
                        po_sb = acts.tile(
                            [DK, NB], f32, tag="posb", bufs=4, name=f"posb{u}_{h}"
                        )
                        nc.vector.tensor_copy(
                            out=po_sb[:], in_=po[0:DK, h * NB:(h + 1) * NB]
                        )
                        po_sbs.append(po_sb)
                        nc.vector.tensor_copy(
                            out=sums_sb[32 * h:32 * h + 1, :],
                            in_=po[DK:DK + 1, h * NB:(h + 1) * NB],
                        )
                    rcr = acts.tile(
                        [33, NB], f32r, tag="rcr", bufs=2, name=f"rcr{u}"
                    )
                    with nc.allow_low_precision("fp22 recip feeds f32r matmul"):
                        nc.vector.reciprocal(out=rcr[:], in_=sums_sb[:])
                    state["pending"] = (u, rcr, po_sbs)

                def emit_step(u, j):
                    # one attention j-step of unit u
                    ib, p = units[u]
                    if j == 0:
                        state["po"] = long_(f"po{u}")
                    po = state["po"]
                    sc = sc_q.popleft()
                    ex = acts.tile(
                        [P, 2 * NB], bf16, tag="ex", bufs=3, name=f"ex{u}_{j}"
                    )
                    nc.scalar.activation(ex[:], sc[:], Exp, scale=0.125)
                    nj = j + 2
                    if nj < NST:
                        sc_q.append(emit_scores(u, nj))
                    elif u + 1 < 8:
                        sc_q.append(emit_scores(u + 1, nj - NST))
                    for h in range(2):
                        nc.tensor.matmul(
                            po[0:DK + 1, h * NB:(h + 1) * NB],
                            va[:, j, 2 * p + h, :],
                            ex[:, h * NB:(h + 1) * NB],
                            start=(j == 0),
                            stop=(j == NST - 1),
                        )
                    # spread previous-unit finish work across this j-loop
                    if state["pending"] is not None and j == 5:
                        pu, rcr, po_sbs = state["pending"]
                        emit_bc(pu, rcr, po_sbs)
                        if units[pu][1] == 1:
                            for t in range(NB // P):
                                fin_q.append((units[pu][0], t))
                        state["pending"] = None
                    elif j in (8, 12) and fin_q:
                        emit_fin(*fin_q.popleft())
                    if j == NST - 1:
                        emit_unit_end(u)

                # ---------------- fused emission ----------------------------
                # phase 1 with unit 0's first 10 j-steps interleaved (lag 6)
                for it in range(NST):
                    if it >= 6:
                        emit_step(0, it - 6)
                    emit_vproj(it)
                    if it % 4 == 3:
                        emit_qk(it // 4)
                        if it == 3:
                            sc_q.append(emit_scores(0, 0))
                            sc_q.append(emit_scores(0, 1))
                # unit 0 continues from j=10, then units 1..7
                for j in range(10, NST):
                    emit_step(0, j)
                for u in range(1, 8):
                    for j in range(NST):
                        emit_step(u, j)
                # tail: finish of the last unit
                pu, rcr, po_sbs = state["pending"]
                emit_bc(pu, rcr, po_sbs)
                for t in range(NB // P):
                    emit_fin(units[pu][0], t)
                while fin_q:
                    emit_fin(*fin_q.popleft())

    _split_excess_waits(nc)
    return nc


def _get_program():
    if "nc" not in _CACHE:
        _CACHE["nc"] = _build_program()
    return _CACHE["nc"]


def kernel(x, Wq, bq, Wk, bk, Wv, bv, Wo, bo, _trace=False):
    import ml_dtypes
    from concourse.bass_utils import run_bass_kernel_spmd

    bft = np.dtype(ml_dtypes.bfloat16)
    x = np.asarray(x, dtype=np.float32)
    Wq = np.asarray(Wq, dtype=np.float32)
    Wk = np.asarray(Wk, dtype=np.float32)
    Wv = np.asarray(Wv, dtype=np.float32)
    Wo = np.asarray(Wo, dtype=np.float32)
    bq = np.asarray(bq, dtype=np.float32)
    bk = np.asarray(bk, dtype=np.float32)
    bv = np.asarray(bv, dtype=np.float32)
    bo = np.asarray(bo, dtype=np.float32)

    in_maps = []
    for c in range(NCORES):
        b = c // 4
        cs = (c % 4) * HD
        in_maps.append({
            "xbh": np.ascontiguousarray(x[b].astype(bft)),
            "wq": np.ascontiguousarray(Wq[:, cs:cs + HD].astype(bft)),
            "wk": np.ascontiguousarray(Wk[:, cs:cs + HD].astype(bft)),
            "wv": np.ascontiguousarray(Wv[:, cs:cs + HD].astype(bft)),
            "wo": np.ascontiguousarray(Wo[cs:cs + HD, :]),
            "bqt": np.ascontiguousarray(bq[cs:cs + HD].reshape(2, P).T),
            "bkt": np.ascontiguousarray(bk[cs:cs + HD].reshape(2, P).T),
        })

    nc = _get_program()
    res = run_bass_kernel_spmd(
        nc, in_maps, core_ids=list(range(NCORES)), trace=_trace
    )

    cvec = (bv @ Wo + bo).astype(np.float32)
    out = np.empty((B, S, D), dtype=np.float32)
    for b in range(B):
        acc = res.results[4 * b]["outp"].astype(np.float64)
        for c in range(4 * b + 1, 4 * b + 4):
            acc = acc + res.results[c]["outp"]
        out[b] = (acc + cvec).astype(np.float32)

    if _trace:
        _CACHE["last_results"] = res
    return out
